# revision 40
# baseline (speedup 1.0000x reference)
"""ChebNet (K=4, 2 ChebConv layers + relu + log_softmax) on 8 trn2 NeuronCores.

Strategy (graph/data parallel, dense-ified SpMM on the TensorEngine):
  - The scaled-Laplacian propagation  prop(h) = A @ h  with
    A = -D^-1/2 Adj D^-1/2  is factored as  A = -diag(dis) @ Cnt @ diag(dis)
    where Cnt is the (dense-ified) edge-count matrix whose entries are small
    integers -- exactly representable in fp8e4m3.
  - Nodes are sharded 8 ways by destination. Each core keeps its Cnt^T shard
    [src=N_pad, dst=DLOC] fully SBUF-resident in fp8 (~12.6 MB) and computes
    prop outputs for its 1/8 of the nodes as a chain of fp8 DoubleRow PE
    matmuls (2 fp8 weights per cell -> 256-src contraction per pass):
    stationary lhsT = fp8 tile-pairs of dis*T_{k-1} (the all-gathered "g"),
    moving rhs = fp8 Cnt^T column pairs, accumulated fp32 in PSUM.
  - Layer 1 runs the standard T-recursion; the per-core [DLOC, F] fp8 shards
    of dis*T_k are exchanged with chunked 8-core AllGathers.
  - Layer 2 uses a Horner rewrite with host-folded weights:
      out = (h@Wc) + A[(h@Wb1) + A[(h@Wb2) + A(h@Wa)]] + b2,
      Wa=4*W2[3], Wb2=2*W2[2], Wb1=W2[1]-3*W2[3], Wc=W2[0]-W2[2]
    so only 16-wide u vectors are propagated/exchanged (3 tiny AllGathers;
    the big h exchange disappears entirely).
  - Per-order weight matmuls run as f32r (1 cycle/row) on the PE; the
    recursion / relu / bias / log_softmax run in fp32 on DVE/ACT.

Everything is computed in a feature-transposed layout [F, node] per core.
"""

import sys

sys.path.insert(0, "/opt/trn_rl_repo")

import numpy as np
import ml_dtypes

import concourse.bacc as bacc
import concourse.mybir as mybir
import concourse.tile as tile
from concourse.bass_utils import run_bass_kernel_spmd
from concourse.masks import make_identity

F32 = mybir.dt.float32
BF16 = mybir.dt.bfloat16
F8E4 = mybir.dt.float8e4
F32R = mybir.dt.float32r
DR = mybir.MatmulPerfMode.DoubleRow

NCORES = 8
P = 128

# full-size problem geometry
N = 10000
F_IN = 128
HID = 128
C_OUT = 16
K_ORD = 4


class Geom:
    """Problem geometry. tiles_per_core src-tiles of 128 nodes per core."""

    def __init__(self, n_nodes, tiles_per_core, f_in=F_IN, hid=HID, c_out=C_OUT,
                 k_ord=K_ORD):
        self.n = n_nodes
        self.tpc = tiles_per_core          # src tiles per core (DLOC/128)
        self.dloc = tiles_per_core * P     # nodes per core (padded)
        self.npad = self.dloc * NCORES     # padded node count
        self.nt = self.npad // P           # total src tiles
        self.f = f_in
        self.hid = hid
        self.c = c_out
        self.k = k_ord
        assert self.npad >= n_nodes
        assert f_in == P and hid == P
        # psum chunking of the dloc free dim (max 512 fp32 per bank)
        self.chunks = []
        off = 0
        while off < self.dloc:
            sz = min(512, self.dloc - off)
            self.chunks.append((off, sz))
            off += sz
        # per-chunk local tile ranges (for the chunked exchange)
        self.ctiles = [(off // P, (off + sz) // P) for off, sz in self.chunks]
        # a-tile groups (DMA/residency granularity): 8 src tiles per group
        self.ag = 8
        assert self.nt % self.ag == 0
        self.n_agrp = self.nt // self.ag
        # g-tile groups: tpc src tiles per group (one core's shard)
        self.n_ggrp = NCORES
        # DoubleRow pairing: every chunk group must hold an even tile count
        for t0, t1 in self.ctiles:
            assert (t1 - t0) % 2 == 0, "chunk groups must pair evenly"


FULL = Geom(N, 10)  # 1280 nodes/core, npad=10240, 80 src tiles

# PE-warming filler matmuls issued while an AllGather is in flight, so the
# HAM clock gate stays at 8/8 for the next propagation. Each is a [128,256]
# matmul (~107 ns warm). Counts are tuned so the warms drain just as the
# AllGather lands -- on the in-order PE, excess warms DELAY the next prop.
N_WARM_STEP = 20
N_WARM_USTEP = 24
N_WARM_START = 30
N_WARM_FIRST = 20


def build_nc(g: Geom):
    nc = bacc.Bacc("TRN2", target_bir_lowering=False, debug=False,
                   num_devices=NCORES)

    # ---- kernel I/O ----------------------------------------------------
    # fp8 Cnt^T shard, split per psum-chunk: [n_agrp, 128, ag, chunk_sz]
    a_in = [nc.dram_tensor(f"a_in_c{ci}", [g.n_agrp, P, g.ag, sz], F8E4,
                           kind="ExternalInput")
            for ci, (off, sz) in enumerate(g.chunks)]
    # initial g = dis * x, tiled [128, nt, f], fp8
    g0_in = nc.dram_tensor("g0_in", [P, g.nt, g.f], F8E4, kind="ExternalInput")
    # local x^T shard fp32 (T0 in transposed layout)
    xt_in = nc.dram_tensor("xt_in", [P, g.dloc], F32, kind="ExternalInput")
    # broadcast +dis rows for the local shard
    disp_in = nc.dram_tensor("disp_in", [P, g.dloc], F32, kind="ExternalInput")
    w1_in = nc.dram_tensor("w1_in", [P, g.k, g.hid], BF16,
                           kind="ExternalInput")
    # host-folded Horner weights: [Wa, Wb2, Wb1, Wc] stacked [P, 4, c]
    w2h_in = nc.dram_tensor("w2h_in", [P, 4, g.c], BF16, kind="ExternalInput")
    # bias columns: col 0 = b1 (hid rows), col 1 = b2 (c rows)
    bb_in = nc.dram_tensor("bb_in", [P, 2], F32, kind="ExternalInput")

    out_dram = nc.dram_tensor("out", [g.dloc, g.c], F32, kind="ExternalOutput")

    n_ag1 = 2  # chunked fp8 allgathers: L1 T1, L1 T2
    n_ag2 = 3  # one-shot u allgathers:  u3, u2, u1
    groups = [list(range(NCORES))]

    with tile.TileContext(nc) as tc:
        with (
            tc.tile_pool(name="pers", bufs=1) as pers,
            tc.tile_pool(name="work", bufs=1) as work,
            tc.tile_pool(name="psum", bufs=1, space="PSUM") as psp,
            tc.tile_pool(name="dram", bufs=1, space="DRAM") as drp,
        ):
            # ---- persistent SBUF ---------------------------------------
            a_sb = [[pers.tile([P, g.ag, sz], F8E4, tag=f"a{ci}_{i}",
                                name=f"a{ci}_{i}")
                     for i in range(g.n_agrp)]
                    for ci, (off, sz) in enumerate(g.chunks)]
            gbufG = [[pers.tile([P, NCORES, t1 - t0, g.f], F8E4,
                                tag=f"g{b}_{ci}", name=f"g{b}_{ci}")
                      for ci, (t0, t1) in enumerate(g.ctiles)]
                     for b in range(2)]
            ubuf = [pers.tile([P, NCORES, g.tpc, g.c], F8E4, tag=f"u{b}",
                              name=f"u{b}")
                    for b in range(2)]
            t_sb = [pers.tile([P, g.dloc], F32, tag=f"t{i}", name=f"t{i}")
                    for i in range(3)]
            disp = pers.tile([P, g.dloc], F32, name="disp")
            acc = pers.tile([P, g.dloc], F32, name="acc")
            gcast = pers.tile([P, g.dloc], BF16, name="gcast")
            hb = pers.tile([P, g.dloc], BF16, name="hb")
            tstage = pers.tile([P, g.tpc, g.f], F8E4, name="tstage")
            ustage = pers.tile([P, g.tpc, g.c], F8E4, name="ustage")
            w1_sb = pers.tile([P, g.k, g.hid], BF16, name="w1_sb")
            w2h_sb = pers.tile([P, 4, g.c], BF16, name="w2h_sb")
            bb_sb = pers.tile([P, 2], F32, name="bb_sb")
            idf32 = pers.tile([P, P], F32, name="idf32")
            idbf = pers.tile([P, P], BF16, name="idbf")

            # ---- DRAM bounce buffers for the collectives ---------------
            # partition-major payload: row = SBUF partition (node-in-tile),
            # col = (tile, feature).
            # per-step L1 exchange in two pieces: chunk 0 alone (issued as
            # soon as it is staged) and chunks 1.. merged.
            tcA = g.ctiles[0][1]          # tiles in chunk 0
            tcB = g.tpc - tcA             # tiles in the rest
            ag_srcA = [drp.tile([P, tcA * g.f], F8E4, name=f"ag_srca{i}")
                       for i in range(n_ag1)]
            ag_dstA = [drp.tile([NCORES * P, tcA * g.f], F8E4,
                                addr_space="Shared", name=f"ag_dsta{i}")
                       for i in range(n_ag1)]
            ag_srcB = [drp.tile([P, max(tcB, 1) * g.f], F8E4,
                                name=f"ag_srcb{i}")
                       for i in range(n_ag1)]
            ag_dstB = [drp.tile([NCORES * P, max(tcB, 1) * g.f], F8E4,
                                addr_space="Shared", name=f"ag_dstb{i}")
                       for i in range(n_ag1)]
            ag_srcU = [drp.tile([P, g.tpc * g.c], F8E4, name=f"ag_srcu{s}")
                       for s in range(n_ag2)]
            ag_dstU = [drp.tile([NCORES * P, g.tpc * g.c], F8E4,
                                addr_space="Shared", name=f"ag_dstu{s}")
                       for s in range(n_ag2)]
            wu_src = drp.tile([P, 2], F32, name="wu_src")
            wu_dst = drp.tile([NCORES * P, 2], F32, addr_space="Shared",
                              name="wu_dst")

            # ---- CC warmup: trigger a tiny AllGather as early as possible
            # (input via a 1KB DRAM->DRAM bounce, no compute dependencies)
            # so the collective-subsystem init overlaps the A load instead
            # of stalling the first real exchange.
            nc.sync.dma_start(wu_src[:], bb_in.ap())
            nc.gpsimd.collective_compute(
                "AllGather", mybir.AluOpType.bypass, replica_groups=groups,
                ins=[wu_src[:]], outs=[wu_dst[:]])

            make_identity(nc, idf32[:])
            make_identity(nc, idbf[:])

            # ---- loads, spread over three DGE queues so the fixed costs
            # overlap; g0 + small tensors first so PE can start early.
            dges = [nc.sync, nc.scalar, nc.gpsimd]
            # g0 is stored group-major on the host: one DMA per chunk-group
            goff = 0
            for ci, (t0, t1) in enumerate(g.ctiles):
                gt = t1 - t0
                dges[ci % 3].dma_start(
                    gbufG[0][ci][:],
                    g0_in[:, goff:goff + NCORES * gt, :]
                    .rearrange("p (j t) f -> p j t f", j=NCORES))
                goff += NCORES * gt
            ld = 0
            nc.sync.dma_start(t_sb[0][:], xt_in[:])
            nc.scalar.dma_start(disp[:], disp_in[:])
            nc.scalar.dma_start(w1_sb[:], w1_in[:])
            nc.scalar.dma_start(w2h_sb[:], w2h_in[:])
            nc.scalar.dma_start(bb_sb[:], bb_in[:])
            # chunk-0 slices of A first: the first propagation's chunk 0
            # can start as soon as they land.
            for ci in range(len(g.chunks)):
                for i in range(g.n_agrp):
                    dges[ld % 3].dma_start(a_sb[ci][i][:], a_in[ci][i])
                    ld += 1

            def warm(n_mm, rhs_ap):
                """Keep the PE HAM clock gate open with dummy matmuls."""
                for _ in range(n_mm):
                    wp = psp.tile([P, 256], F32, space="PSUM", tag="warm",
                                  name="wp")
                    nc.tensor.matmul(wp[:, :rhs_ap.free_size()], lhsT=idbf[:],
                                     rhs=rhs_ap, start=True, stop=True,
                                     skip_group_check=True)

            warm(N_WARM_START, gbufG[0][0][:, 0, 0:1, :])

            ag_idx = 0
            cur = 0  # g-buffer ping-pong index; gbuf[0] holds g(x)

            def chunk_tiles(off, sz):
                return range(off // P, (off + sz) // P)

            # global src-tile-PAIR order, chunk-group-major (earliest
            # AllGather chunk first) to match exchange arrival order.
            pair_order = [(gci, j, q)
                          for gci, (t0, t1) in enumerate(g.ctiles)
                          for j in range(g.n_ggrp)
                          for q in range((t1 - t0) // 2)]
            n_pairs = g.nt // 2

            def stage_chunk(idx, ci, t_src, off, sz):
                """fp8 stage of dis*t_src: bf16 mult, transpose, fp8 copy."""
                nc.vector.tensor_tensor(out=gcast[:, off:off + sz],
                                        in0=t_src[:, off:off + sz],
                                        in1=disp[:, off:off + sz],
                                        op=mybir.AluOpType.mult)
                for t in chunk_tiles(off, sz):
                    tpb = psp.tile([P, P], BF16, space="PSUM", tag="tpb",
                                   name="tpb", bufs=2)
                    nc.tensor.transpose(out=tpb[:],
                                        in_=gcast[:, t * P:(t + 1) * P],
                                        identity=idbf[:])
                    nc.vector.tensor_copy(tstage[:, t, :], tpb[:])
                t0, t1 = off // P, (off + sz) // P
                if ci == 0:
                    nc.scalar.dma_start(ag_srcA[idx][:],
                                        tstage[:, 0:tcA, :])
                else:
                    nc.scalar.dma_start(
                        ag_srcB[idx][:, (t0 - tcA) * g.f:(t1 - tcA) * g.f],
                        tstage[:, t0:t1, :])

            def allgather_first(idx, b_next):
                """Exchange chunk 0 as soon as it is staged."""
                nc.gpsimd.collective_compute(
                    "AllGather",
                    mybir.AluOpType.bypass,
                    replica_groups=groups,
                    ins=[ag_srcA[idx][:]],
                    outs=[ag_dstA[idx][:]],
                )
                nc.sync.dma_start(
                    gbufG[b_next][0][:],
                    ag_dstA[idx][:]
                    .rearrange("(j p) (t f) -> p j t f", p=P, f=g.f))

            def allgather_rest(idx, b_next):
                """Exchange the remaining chunks in one collective."""
                if tcB == 0:
                    return
                nc.gpsimd.collective_compute(
                    "AllGather",
                    mybir.AluOpType.bypass,
                    replica_groups=groups,
                    ins=[ag_srcB[idx][:]],
                    outs=[ag_dstB[idx][:]],
                )
                n_w = N_WARM_FIRST if idx == 0 else N_WARM_STEP
                warm(n_w, gcast[:, 0:min(256, g.dloc)])
                dstb = (ag_dstB[idx][:]
                        .rearrange("(j p) (t f) -> p j t f", p=P, f=g.f))
                for ci in range(1, len(g.ctiles)):
                    t0, t1 = g.ctiles[ci]
                    dges[ci % 3].dma_start(
                        gbufG[b_next][ci][:],
                        dstb[:, :, t0 - tcA:t1 - tcA, :])

            def w_term_chunk(k, t_src, off, sz, first, tb_eng=None):
                """acc[:, chunk] (+)= (T_k @ W1[k])^T, bf16 matmul (an idle
                engine makes a bf16 copy of the T chunk first)."""
                tb = work.tile([P, 512], BF16, tag="tb", name="tb", bufs=2)
                if tb_eng is None:
                    nc.scalar.activation(
                        tb[:, :sz], t_src[:, off:off + sz],
                        mybir.ActivationFunctionType.Identity)
                else:
                    tb_eng.tensor_copy(tb[:, :sz], t_src[:, off:off + sz])
                wt = psp.tile([P, 512], F32, space="PSUM", tag="wt",
                              name="wt", bufs=2)
                nc.tensor.matmul(
                    wt[:, :sz],
                    lhsT=w1_sb[:, k, :],
                    rhs=tb[:, :sz],
                    start=True, stop=True,
                )
                if first:
                    nc.vector.tensor_copy(acc[:, off:off + sz], wt[:, :sz])
                else:
                    nc.vector.tensor_add(acc[:, off:off + sz],
                                         acc[:, off:off + sz], wt[:, :sz])

            def y_term_chunk(slot, off, sz):
                """wt16 = (h @ W2h[slot])^T for this chunk, bf16 matmul
                against the persistent bf16 copy of h."""
                wt = psp.tile([P, 512], F32, space="PSUM", tag="wt",
                              name="wt", bufs=2)
                nc.tensor.matmul(
                    wt[:g.c, :sz],
                    lhsT=w2h_sb[:, slot, :],
                    rhs=hb[:, off:off + sz],
                    start=True, stop=True,
                )
                return wt

            def stage_u_chunk(s, u_src, off, sz):
                """fp8 stage of dis*u (16-wide) into ustage + ag_srcU[s]."""
                nc.vector.tensor_tensor(out=gcast[:g.c, off:off + sz],
                                        in0=u_src[:g.c, off:off + sz],
                                        in1=disp[:g.c, off:off + sz],
                                        op=mybir.AluOpType.mult)
                for t in chunk_tiles(off, sz):
                    tpu = psp.tile([P, g.c], BF16, space="PSUM", tag="tpb",
                                   name="tpu", bufs=2)
                    nc.tensor.transpose(out=tpu[:],
                                        in_=gcast[:g.c, t * P:(t + 1) * P],
                                        identity=idbf[:g.c, :g.c])
                    nc.vector.tensor_copy(ustage[:, t, :], tpu[:])
                t0, t1 = off // P, (off + sz) // P
                nc.scalar.dma_start(
                    ag_srcU[s][:, t0 * g.c:t1 * g.c],
                    ustage[:, t0:t1, :])

            def allgather_u(s, b_next):
                nc.gpsimd.collective_compute(
                    "AllGather",
                    mybir.AluOpType.bypass,
                    replica_groups=groups,
                    ins=[ag_srcU[s][:]],
                    outs=[ag_dstU[s][:]],
                )
                warm(N_WARM_USTEP, gcast[:, 0:min(256, g.dloc)])
                nc.sync.dma_start(
                    ubuf[b_next][:],
                    ag_dstU[s][:]
                    .rearrange("(j p) (t c) -> p j t c", p=P, c=g.c))

            z_all = work.tile([P, g.tpc, g.c], F32, name="z_all")
            m_all = work.tile([P, g.tpc, 1], F32, name="m_all")
            e_all = work.tile([P, g.tpc, g.c], F32, name="e_all")
            s_all = work.tile([P, g.tpc, 1], F32, name="s_all")
            o_all = work.tile([P, g.tpc, g.c], F32, name="o_all")
            out_ap = out_dram.ap().rearrange("(t p) c -> p t c", p=P)

            def z_prep_chunk(off, sz):
                """Per-chunk DVE part of log_softmax: rowmax + shift."""
                t0, t1 = off // P, (off + sz) // P
                ct = t1 - t0
                zs = z_all[:, t0:t1, :]
                nc.vector.tensor_reduce(out=m_all[:, t0:t1, 0], in_=zs,
                                        axis=mybir.AxisListType.X,
                                        op=mybir.AluOpType.max)
                nc.vector.tensor_tensor(out=e_all[:, t0:t1, :], in0=zs,
                                        in1=m_all[:, t0:t1, :].to_broadcast(
                                            [P, ct, g.c]),
                                        op=mybir.AluOpType.subtract)

            # ================= Layer 1: T-recursion =====================
            for (off, sz) in g.chunks:
                w_term_chunk(0, t_sb[0], off, sz, first=True)

            for k in range(1, g.k):
                tk = t_sb[k % 3]
                tk2 = t_sb[(k - 2) % 3] if k >= 2 else None
                do_stage = k < g.k - 1  # T3 needs no exchange

                def tail_chunk(ci, off, sz, k=k, tk=tk):
                    """W-term + staging for a finished chunk. Emitted a few
                    matmuls into the NEXT chunk so the in-order PE never
                    stalls on the DVE recursion results."""
                    if do_stage:
                        w_term_chunk(k, tk, off, sz, first=False)
                        stage_chunk(ag_idx, ci, tk, off, sz)
                        if ci == 0:
                            allgather_first(ag_idx, 1 - cur)
                        if ci == len(g.chunks) - 1:
                            allgather_rest(ag_idx, 1 - cur)
                    else:
                        # layer end: h = relu(acc + b1) -> t_sb[0] and a
                        # second relu into bf16 hb (no ACT-table thrash),
                        # then u3 = h @ Wa staged. The tb copy goes to the
                        # vector engine to keep scalar on Relu only.
                        w_term_chunk(k, tk, off, sz, first=False,
                                     tb_eng=nc.gpsimd)
                        nc.scalar.activation(
                            t_sb[0][:, off:off + sz],
                            acc[:, off:off + sz],
                            mybir.ActivationFunctionType.Relu,
                            bias=bb_sb[:, 0:1], scale=1.0)
                        nc.scalar.activation(
                            hb[:, off:off + sz],
                            acc[:, off:off + sz],
                            mybir.ActivationFunctionType.Relu,
                            bias=bb_sb[:, 0:1], scale=1.0)
                        wt = y_term_chunk(0, off, sz)  # Wa = 4*W2[3]
                        nc.vector.tensor_copy(acc[:g.c, off:off + sz],
                                              wt[:g.c, :sz])
                        stage_u_chunk(0, acc, off, sz)

                pending = None
                for ci, (off, sz) in enumerate(g.chunks):
                    pp = psp.tile([P, 512], F32, space="PSUM", tag="pp",
                                  name="pp", bufs=2)
                    for n_i, (gci, j, q) in enumerate(pair_order):
                        t0g = g.ctiles[gci][0]
                        gi = j * g.tpc + t0g + 2 * q
                        lhs = gbufG[cur][gci][:, j, 2 * q:2 * q + 2, :]
                        nc.tensor.matmul(
                            pp[:, :sz],
                            lhsT=lhs,
                            rhs=a_sb[ci][gi // g.ag][:, gi % g.ag:
                                                     gi % g.ag + 2, :],
                            start=(n_i == 0),
                            stop=(n_i == n_pairs - 1),
                            perf_mode=DR,
                        )
                        if n_i == 8 and pending is not None:
                            pending()
                            pending = None
                    # Chebyshev recursion (fp32, on DVE)
                    if k == 1:
                        nc.vector.scalar_tensor_tensor(
                            out=tk[:, off:off + sz],
                            in0=pp[:, :sz],
                            scalar=-1.0,
                            in1=disp[:, off:off + sz],
                            op0=mybir.AluOpType.mult,
                            op1=mybir.AluOpType.mult)
                    else:
                        nc.vector.scalar_tensor_tensor(
                            out=tk[:, off:off + sz],
                            in0=pp[:, :sz],
                            scalar=-2.0,
                            in1=disp[:, off:off + sz],
                            op0=mybir.AluOpType.mult,
                            op1=mybir.AluOpType.mult)
                        nc.vector.tensor_sub(
                            tk[:, off:off + sz],
                            tk[:, off:off + sz],
                            tk2[:, off:off + sz])
                    pending = (lambda ci=ci, off=off, sz=sz:
                               tail_chunk(ci, off, sz))
                pending()
                if do_stage:
                    ag_idx += 1
                    cur = 1 - cur

            # u3 staged by the k=3 tails; exchange it.
            allgather_u(0, 0)

            # ================= Layer 2: Horner on 16-wide u =============
            # step s: v = A @ u_in;  u_out = v + h @ W2h[s+1]
            # s=0: u3 -> u2 (Wb2)   s=1: u2 -> u1 (Wb1)   s=2: u1 -> z (Wc)
            for s in range(3):
                ub = ubuf[s % 2]
                last_step = s == 2

                def tail_u(ci, off, sz, s=s):
                    wt = y_term_chunk(s + 1, off, sz)
                    nc.vector.tensor_add(acc[:g.c, off:off + sz],
                                         acc[:g.c, off:off + sz],
                                         wt[:g.c, :sz])
                    if not last_step:
                        stage_u_chunk(s + 1, acc, off, sz)
                    else:
                        # final: + b2 (on DVE), transpose to node-major z
                        nc.vector.tensor_add(
                            acc[:g.c, off:off + sz],
                            acc[:g.c, off:off + sz],
                            bb_sb[:g.c, 1:2].to_broadcast([g.c, sz]))
                        for t in chunk_tiles(off, sz):
                            zp = psp.tile([P, g.c], F32, space="PSUM",
                                          tag="tpb", name="zp", bufs=2)
                            nc.tensor.transpose(
                                out=zp[:],
                                in_=acc[:g.c, t * P:(t + 1) * P],
                                identity=idf32[:g.c, :g.c])
                            nc.vector.tensor_copy(z_all[:, t, :], zp[:])
                        z_prep_chunk(off, sz)

                pending = None
                for ci, (off, sz) in enumerate(g.chunks):
                    pp = psp.tile([P, 512], F32, space="PSUM", tag="pp",
                                  name="pp", bufs=2)
                    for n_i in range(n_pairs):
                        gi = 2 * n_i
                        nc.tensor.matmul(
                            pp[:g.c, :sz],
                            lhsT=ub[:, gi // g.tpc, gi % g.tpc:
                                    gi % g.tpc + 2, :],
                            rhs=a_sb[ci][gi // g.ag]
                                    [:, gi % g.ag:gi % g.ag + 2, :],
                            start=(n_i == 0),
                            stop=(n_i == n_pairs - 1),
                            perf_mode=DR,
                        )
                        if n_i == 8 and pending is not None:
                            pending()
                            pending = None
                    # u_out chunk = -dis * pp  (the A part of v)
                    nc.vector.scalar_tensor_tensor(
                        out=acc[:g.c, off:off + sz],
                        in0=pp[:g.c, :sz],
                        scalar=-1.0,
                        in1=disp[:g.c, off:off + sz],
                        op0=mybir.AluOpType.mult,
                        op1=mybir.AluOpType.mult)
                    pending = (lambda ci=ci, off=off, sz=sz:
                               tail_u(ci, off, sz))
                pending()
                if not last_step:
                    allgather_u(s + 1, (s + 1) % 2)

            # batched scalar part of log_softmax (2 ACT-table loads total)
            nc.scalar.activation(e_all[:], e_all[:],
                                 mybir.ActivationFunctionType.Exp)
            nc.vector.tensor_reduce(out=s_all[:, :, 0], in_=e_all[:],
                                    axis=mybir.AxisListType.X,
                                    op=mybir.AluOpType.add)
            nc.scalar.activation(s_all[:], s_all[:],
                                 mybir.ActivationFunctionType.Ln)
            nc.vector.tensor_add(s_all[:], s_all[:], m_all[:])
            nc.vector.tensor_tensor(out=o_all[:], in0=z_all[:],
                                    in1=s_all[:].to_broadcast(
                                        [P, g.tpc, g.c]),
                                    op=mybir.AluOpType.subtract)
            nc.sync.dma_start(out_ap[:], o_all[:])

    nc.compile()
    return nc


def host_prep(g: Geom, x, edge_index, W1, b1, W2, b2):
    """Build the per-core input maps (sharding + dense-ification)."""
    n = g.n
    src = np.asarray(edge_index[0], dtype=np.int64)
    dst = np.asarray(edge_index[1], dtype=np.int64)
    deg = np.bincount(src, minlength=n).astype(np.float64)
    dis = np.where(deg > 0, 1.0 / np.sqrt(np.maximum(deg, 1e-12)), 0.0)

    # dense-ified edge-count matrix, transposed: cnt_t[s, d]
    cnt_t = np.zeros((g.npad, g.npad), dtype=np.float32)
    np.add.at(cnt_t, (src, dst), 1.0)

    dis_pad = np.zeros(g.npad, dtype=np.float32)
    dis_pad[:n] = dis.astype(np.float32)
    x_pad = np.zeros((g.npad, g.f), dtype=np.float32)
    x_pad[:n] = np.asarray(x, dtype=np.float32)

    g0 = dis_pad[:, None] * x_pad  # [npad, f]
    g0_tiles = (g0.reshape(g.nt, P, g.f).transpose(1, 0, 2)
                .astype(ml_dtypes.bfloat16).astype(ml_dtypes.float8_e4m3))
    # group-major tile order: for each chunk-group, all cores' tiles
    order = [j * g.tpc + t
             for (t0, t1) in g.ctiles
             for j in range(NCORES)
             for t in range(t0, t1)]
    g0_tiles = g0_tiles[:, order, :]

    w1 = np.ascontiguousarray(
        np.asarray(W1, np.float32).transpose(1, 0, 2)
        .astype(ml_dtypes.bfloat16))  # [P, k, hid]
    W2f = np.asarray(W2, np.float32)
    w2h_stack = np.stack([
        4.0 * W2f[3],            # Wa
        2.0 * W2f[2],            # Wb2
        W2f[1] - 3.0 * W2f[3],   # Wb1
        W2f[0] - W2f[2],         # Wc
    ], axis=0)  # [4, hid, c]
    w2h = np.ascontiguousarray(
        w2h_stack.transpose(1, 0, 2).astype(ml_dtypes.bfloat16))  # [P, 4, c]
    bb = np.zeros((P, 2), np.float32)
    bb[:g.hid, 0] = np.asarray(b1, np.float32)
    bb[:g.c, 1] = np.asarray(b2, np.float32)

    in_maps = []
    for c in range(NCORES):
        lo, hi = c * g.dloc, (c + 1) * g.dloc
        a_c = (cnt_t[:, lo:hi].astype(ml_dtypes.float8_e4m3)
               .reshape(g.n_agrp, g.ag, P, g.dloc).transpose(0, 2, 1, 3))
        a_chunks = [np.ascontiguousarray(a_c[:, :, :, off:off + sz])
                    for (off, sz) in g.chunks]
        xt = np.ascontiguousarray(x_pad[lo:hi].T)          # [128, dloc]
        d_loc = dis_pad[lo:hi]
        disp = np.ascontiguousarray(
            np.broadcast_to(d_loc[None, :], (P, g.dloc))).astype(np.float32)
        im = {f"a_in_c{ci}": a_chunks[ci] for ci in range(len(g.chunks))}
        im.update({
            "g0_in": np.ascontiguousarray(g0_tiles),
            "xt_in": xt,
            "disp_in": disp,
            "w1_in": w1,
            "w2h_in": w2h,
            "bb_in": bb,
        })
        in_maps.append(im)
    return in_maps


_CACHED_NC = None


def _get_nc():
    global _CACHED_NC
    if _CACHED_NC is None:
        _CACHED_NC = build_nc(FULL)
    return _CACHED_NC


def _enable_ldw_opt():
    """The default axon compile flags pass --enable-ldw-opt=false, which
    serializes every LDWEIGHTS with its MATMUL (~+107ns per matmul). Our
    kernel is a long stream of ldweights+matmul pairs, so re-enable it."""
    try:
        from concourse.compiler_utils import (get_compiler_flags,
                                              set_compiler_flags)
        flags = get_compiler_flags()
        new = [f.replace("--enable-ldw-opt=false", "--enable-ldw-opt=true")
               for f in flags]
        if new != flags:
            set_compiler_flags(new)
    except Exception:
        pass


def kernel(x, edge_index, W1, b1, W2, b2, _profile=False):
    g = FULL
    _enable_ldw_opt()
    in_maps = host_prep(g, x, edge_index, W1, b1, W2, b2)
    nc = _get_nc()
    res = run_bass_kernel_spmd(nc, in_maps, list(range(NCORES)),
                               trace=_profile)
    out = np.concatenate([res.results[c]["out"] for c in range(NCORES)], 0)
    out = out[:g.n].astype(np.float32)
    if _profile:
        kernel.last_result = res
    return out
